# revision 8
# baseline (speedup 1.0000x reference)
"""BrightnessLoss Trainium2 kernel (raw Bass, 8-core data parallel).

reference:
    V(x)   = max_c(clip(x, 0, 1))        over channel dim (RGB)
    result = mean(|V(pred) - V(target)|) over (N, H, W)

Identities used on device:
    clip(max(r,g,b),0,1) == max_c(clip(x,0,1))          (clip is monotone)
    W := relu(1 - relu(m)) == 1 - clip(m, 0, 1)
    |Vp - Vt| == |Wp - Wt|
    sum|Wp - Wt| == 2*sum max(Wp,Wt) - sum Wp - sum Wt

Per core (8 cores, 4 images each), per unit (image or image-chunk):
    SP     2x dma  [128, 3*fc] f32     pred / target planes (HWDGE, line rate)
    DVE    m1 = max(R,G); u = (m1 max 0) max B   (fused relu, x2 sides)
    ACT    W = Relu(-u + 1), accum_out = sum(W)  (x2 sides; side sums free)
    GPSIMD stt bypass,max: max(Wp,Wt) with accum_out = sum  (off DVE's back)
    SP     dma acc -> partials

Software pipeline over units, double-buffered by unit parity. Per-side DMA
waits so DVE starts right after the pred plane lands. Host combines
per-core partials in float64.
"""

import numpy as np

N_CORES = 8
N_IMG = 4  # 32 / 8
C = 3
P = 128
F = 2048  # 512*512 / 128
N_PIX = 32 * 512 * 512
N_CHUNKS = 2  # chunks per plane (tail-latency lever)
ACCUM_ENGINE = "vector"  # Pool lacks TensorScalarPtr (walrus engine check)


def _build_program(n_img=N_IMG, f=F, n_chunks=N_CHUNKS, accum_engine=ACCUM_ENGINE):
    from contextlib import ExitStack

    import concourse.bass as bass
    import concourse.mybir as mybir

    fp32 = mybir.dt.float32
    Alu = mybir.AluOpType
    Act = mybir.ActivationFunctionType

    assert f % n_chunks == 0
    fc = f // n_chunks
    n_units = n_img * n_chunks

    # detect_race_conditions=False: the raw-mode CoreSim race detector can't
    # see same-engine program-order (DVE m1 -> STT RAW); hardware engines
    # execute in order.
    nc = bass.Bass(
        "TRN2", target_bir_lowering=False, debug=False, detect_race_conditions=False
    )
    pred = nc.dram_tensor("pred", [n_img, C, P, f], fp32, kind="ExternalInput").ap()
    targ = nc.dram_tensor("target", [n_img, C, P, f], fp32, kind="ExternalInput").ap()
    out = nc.dram_tensor(
        "partials", [P, 3 * n_units], fp32, kind="ExternalOutput"
    ).ap()

    def unit_src(side, u):
        # dram AP [P, C, fc] for image u//n_chunks, chunk u%n_chunks
        n, j = divmod(u, n_chunks)
        return side[n, :, :, j * fc : (j + 1) * fc].rearrange("c p f -> p c f")

    with ExitStack() as ctx:
        sb = lambda name, shape: ctx.enter_context(nc.sbuf_tensor(name, shape, fp32))
        sem = lambda name: ctx.enter_context(nc.semaphore(name))

        inb = [[sb(f"in{sl}{s}", [P, C * fc]) for s in range(2)] for sl in range(2)]
        ub = [[sb(f"u{sl}{s}", [P, fc]) for s in range(2)] for sl in range(2)]
        wb = [[sb(f"w{sl}{s}", [P, fc]) for s in range(2)] for sl in range(2)]
        m1 = sb("m1", [P, fc])
        scr = sb("stt_scratch", [P, fc])
        acc = sb("acc", [P, 3 * n_units])

        in_sem = [sem("in0"), sem("in1")]
        u_sem = sem("u")
        act_sem = sem("act")
        gp_sem = sem("gp")
        out_sem = sem("outd")

        block = ctx.enter_context(nc.Block())

        @block.sync
        def _(sync):
            for u in range(n_units):
                if u >= 2:
                    # WAR on inb[u%2]: unit u-2's STTs (last input readers)
                    sync.wait_ge(u_sem, 2 * (u - 1))
                for s, side in enumerate((pred, targ)):
                    sync.dma_start(
                        out=inb[u % 2][s][:].rearrange("p (c f) -> p c f", c=C),
                        in_=unit_src(side, u),
                    ).then_inc(in_sem[u % 2], 16)
            sync.wait_ge(gp_sem, n_units)
            sync.dma_start(out=out[:], in_=acc[:]).then_inc(out_sem, 16)
            sync.wait_ge(out_sem, 16)

        accum_eng = nc.gpsimd if accum_engine == "gpsimd" else nc.vector

        def emit_accum(eng, u):
            # max(Wp, Wt) elementwise, accum_out = per-partition sum
            eng.wait_ge(act_sem, 2 * (u + 1))
            eng.scalar_tensor_tensor(
                scr[:],
                wb[u % 2][0][:],
                0.0,
                wb[u % 2][1][:],
                op0=Alu.bypass,
                op1=Alu.max,
                accum_out=acc[:, 3 * u : 3 * u + 1],
            ).then_inc(gp_sem, 1)

        @block.vector
        def _(vector):
            for u in range(n_units):
                for s in range(2):
                    # pred side ready after 1st dma (16), targ after 2nd (32)
                    vector.wait_ge(in_sem[u % 2], 32 * (u // 2) + 16 * (s + 1))
                    t = inb[u % 2][s]
                    vector.tensor_max(m1[:], t[:, 0:fc], t[:, fc : 2 * fc])
                    if u >= 2:
                        # WAR on ub[u%2][s]: ACT's W of unit u-2 (its reader)
                        vector.wait_ge(act_sem, 2 * (u - 1))
                    vector.scalar_tensor_tensor(
                        ub[u % 2][s][:],
                        m1[:],
                        0.0,
                        t[:, 2 * fc : 3 * fc],
                        op0=Alu.max,
                        op1=Alu.max,
                    ).then_inc(u_sem, 1)
                if accum_engine == "vector" and u > 0:
                    emit_accum(vector, u - 1)
            if accum_engine == "vector":
                emit_accum(vector, n_units - 1)

        if accum_engine == "gpsimd":

            @block.gpsimd
            def _(gpsimd):
                for u in range(n_units):
                    emit_accum(gpsimd, u)

        @block.scalar
        def _(scalar):
            for n in range(n_units):
                for s in range(2):
                    scalar.wait_ge(u_sem, 2 * n + s + 1)
                    if n >= 2:
                        # WAR on wb[n%2][s]: accum of unit n-2 (its reader)
                        scalar.wait_ge(gp_sem, n - 1)
                    scalar.activation(
                        wb[n % 2][s][:],
                        ub[n % 2][s][:],
                        Act.Relu,
                        bias=1.0,
                        scale=-1.0,
                        accum_out=acc[:, 3 * n + 1 + s : 3 * n + 2 + s],
                    ).then_inc(act_sem, 1)

    return nc


_program = None


def _get_program():
    global _program
    if _program is None:
        _program = _build_program()
    return _program


def _finish(partials_list):
    """partials_list: per-core [P, 3*n_units] f32 with cols per unit:
    [sum max(Wp,Wt), sum Wp, sum Wt].
    sum|Vp-Vt| = 2*sum(max) - sum(Wp) - sum(Wt)."""
    total = np.float64(0.0)
    for p in partials_list:
        p = p.astype(np.float64)
        total += 2.0 * p[:, 0::3].sum() - p[:, 1::3].sum() - p[:, 2::3].sum()
    return np.array(total / N_PIX, dtype=np.float32)


def kernel(pred: np.ndarray, target: np.ndarray) -> np.ndarray:
    from concourse.bass_utils import run_bass_kernel_spmd

    nc = _get_program()
    pred = np.ascontiguousarray(pred, dtype=np.float32).reshape(
        N_CORES, N_IMG, C, P, F
    )
    target = np.ascontiguousarray(target, dtype=np.float32).reshape(
        N_CORES, N_IMG, C, P, F
    )
    in_maps = [{"pred": pred[i], "target": target[i]} for i in range(N_CORES)]
    res = run_bass_kernel_spmd(nc, in_maps, list(range(N_CORES)))
    return _finish([r["partials"] for r in res.results])
